# revision 1
# baseline (speedup 1.0000x reference)
"""Trainium2 Bass kernel for nn_EdgeConvolution (gnn_message_passing).

Math
----
Reference (B=2, N=512, C=128, U=128), adj binary {0,1}:
  masked[b,i,j,:]  = adj[b,i,j] * x[b,i,:]
  a_sel[b,i]       = adj[b,i, xidx[b,i]]
  edging[b,i,j,:]  = [ adj*x_i | adj*(a_sel - adj)*x_i ]
                   = adj[b,i,j] * [ x_i | (a_sel_i - 1)*x_i ]        (adj^2 = adj)
  out[b,i,j,:]     = relu(adj*(u_i + (a_sel_i-1)*v_i) + b),  u = x@W1, v = x@W2
So over j there are only two values per (b,i):
  z1_i = relu(u_i + (a_sel_i-1)*v_i + b)   (edges with adj=1, count k_i)
  z0   = relu(b)                            (edges with adj=0, count N-k_i)
  maxp_i   = max(1[k_i>0]*z1_i, 1[k_i<N]*z0)
  n_i      = k_i*1[any z1_i>0] + (N-k_i)*1[any z0>0]
  avgpool_i = [ k_i*x_i | k_i*(a_sel_i-1)*x_i ] / n_i
Per-core slab: 128 of the 1024 (b,i) rows; w/b replicated.

Implementation: raw Bass (no Tile) to minimize semaphore/barrier overhead.
Engines: SP ring DMAs (adj, xidx, b) + out; ACT ring DMAs (x|xT pack, w) +
per-partition-scale multiplies; PE: bias fold (ones x [b|0] accumulated into
x@[W1|W2]) and the b broadcast; DVE: reductions and the main chain; Pool:
iota/cast/[P,1] scalars. `n` is computed by selecting between the two
possible reciprocals so only one op depends on s1 = any(z1>0).
"""

import numpy as np

B, N, C, U = 2, 512, 128, 128
P = 128          # rows (b,i) per core == SBUF partitions
NCORES = 8
OUTF = U + 2 * C  # 384

_CACHE: dict = {}


def _build_nc():
    import concourse.bacc as bacc
    import concourse.bass as bass
    import concourse.mybir as mybir

    f32 = mybir.dt.float32
    i32 = mybir.dt.int32
    Alu = mybir.AluOpType
    AX = mybir.AxisListType.X
    Act = mybir.ActivationFunctionType

    nc = bacc.Bacc("TRN2", target_bir_lowering=False, debug=False,
                   num_devices=NCORES)

    adj_d = nc.dram_tensor("adj", [P, N], f32, kind="ExternalInput")
    xb_d = nc.dram_tensor("xboth", [P, 2 * C], f32, kind="ExternalInput")
    xidx_d = nc.dram_tensor("xidx", [P, 1], i32, kind="ExternalInput")
    w_d = nc.dram_tensor("w", [2 * C, U], f32, kind="ExternalInput")
    b_d = nc.dram_tensor("b", [1, U], f32, kind="ExternalInput")
    out_d = nc.dram_tensor("out", [P, OUTF], f32, kind="ExternalOutput")

    ctx_tensors = [
        ("adj_t", [P, N], f32), ("xb_t", [P, 2 * C], f32),
        ("wcat_t", [P, 2 * U], f32), ("xidx_t", [P, 1], i32),
        ("brow_t", [1, U], f32), ("ones1", [1, P], f32),
        ("iota_f", [P, N], f32), ("xidx_f", [P, 1], f32),
        ("scr", [P, N], f32), ("zcol", [P, 1], f32), ("wscr", [P, 1], f32),
        ("a_sel", [P, 1], f32), ("k", [P, 1], f32), ("asm1", [P, 1], f32),
        ("t_sb", [P, U], f32), ("zz", [P, U], f32), ("zzb", [P, U], f32),
        ("z1", [P, U], f32),
        ("z1sum", [P, 1], f32), ("z0", [P, U], f32), ("z0sum", [P, 1], f32),
        ("s0", [P, 1], f32), ("nk", [P, 1], f32), ("h0", [P, 1], f32),
        ("h1", [P, 1], f32), ("t2", [P, 1], f32),
        ("s1", [P, 1], f32), ("nn", [P, 1], f32), ("rn", [P, 1], f32),
        ("xcat", [P, 2 * C], f32), ("z0h", [P, U], f32),
        ("out_t", [P, OUTF], f32),
    ]

    from contextlib import ExitStack
    with ExitStack() as ctx:
        t = {}
        for name, shape, dt in ctx_tensors:
            t[name] = ctx.enter_context(nc.sbuf_tensor(name, shape, dt))
        mm = ctx.enter_context(nc.psum_tensor("mm", [P, 2 * U], f32))
        bc = ctx.enter_context(nc.psum_tensor("bc", [P, U], f32))

        dadj = ctx.enter_context(nc.semaphore("dadj"))
        didx = ctx.enter_context(nc.semaphore("didx"))
        db = ctx.enter_context(nc.semaphore("db"))
        dxb = ctx.enter_context(nc.semaphore("dxb"))
        dwc = ctx.enter_context(nc.semaphore("dwc"))
        sini = ctx.enter_context(nc.semaphore("sini"))
        spe = ctx.enter_context(nc.semaphore("spe"))
        sdve = ctx.enter_context(nc.semaphore("sdve"))
        spool = ctx.enter_context(nc.semaphore("spool"))
        sact = ctx.enter_context(nc.semaphore("sact"))
        sz0 = ctx.enter_context(nc.semaphore("sz0"))
        sfin = ctx.enter_context(nc.semaphore("sfin"))
        dout = ctx.enter_context(nc.semaphore("dout"))

        block = ctx.enter_context(nc.Block())

        ap = lambda h: h.ap()

        # Self-waits use all-incs-so-far thresholds: completions on one
        # engine can retire out of order, so `>= total` is the only
        # order-independent guarantee that a specific producer finished.

        @block.gpsimd
        def _(pool):
            nc.gpsimd.memset(ap(t["ones1"]), 1.0)
            nc.gpsimd.memset(ap(t["zcol"]), 0.0)
            pool.drain().then_inc(sini, 1)
            nc.gpsimd.iota(ap(t["iota_f"]), pattern=[[1, N]], base=0,
                           channel_multiplier=0,
                           allow_small_or_imprecise_dtypes=True
                           ).then_inc(spool, 1)                        # ->1
            pool.wait_ge(didx, 16)
            nc.gpsimd.tensor_copy(ap(t["xidx_f"]),
                                  ap(t["xidx_t"])).then_inc(spool, 1)  # ->2
            pool.wait_ge(sdve, 1)            # k ready
            nc.gpsimd.tensor_scalar(out=ap(t["nk"]), in0=ap(t["k"]),
                                    scalar1=-1.0, scalar2=float(N),
                                    op0=Alu.mult,
                                    op1=Alu.add).then_inc(spool, 1)    # ->3
            nc.gpsimd.tensor_scalar(out=ap(t["h0"]), in0=ap(t["k"]),
                                    scalar1=float(N), scalar2=None,
                                    op0=Alu.is_lt).then_inc(spool, 1)  # ->4
            nc.gpsimd.tensor_scalar(out=ap(t["h1"]), in0=ap(t["k"]),
                                    scalar1=0.0, scalar2=None,
                                    op0=Alu.is_gt).then_inc(spool, 1)  # ->5
            pool.wait_ge(sz0, 1)             # z0sum ready
            nc.gpsimd.tensor_scalar(out=ap(t["s0"]), in0=ap(t["z0sum"]),
                                    scalar1=0.0, scalar2=None,
                                    op0=Alu.is_gt).then_inc(spool, 1)  # ->6
            pool.wait_ge(spool, 6)           # nk + s0 visible (all 6)
            nc.gpsimd.tensor_mul(ap(t["t2"]), ap(t["nk"]),
                                 ap(t["s0"])).then_inc(spool, 1)       # ->7

        @block.sync
        def _(sync):
            sync.dma_start(ap(t["adj_t"]), adj_d.ap()).then_inc(dadj, 16)
            sync.dma_start(ap(t["brow_t"]), b_d.ap()).then_inc(db, 16)
            sync.dma_start(ap(t["xidx_t"]), xidx_d.ap()).then_inc(didx, 16)
            sync.wait_ge(sfin, 2)
            sync.dma_start(out_d.ap(), ap(t["out_t"])).then_inc(dout, 16)
            sync.wait_ge(dout, 16)

        @block.scalar
        def _(act):
            act.dma_start(ap(t["xb_t"]), xb_d.ap()).then_inc(dxb, 16)
            act.dma_start(
                t["wcat_t"].ap().rearrange("p (s u) -> p s u", s=2),
                w_d.ap().rearrange("(s c) u -> c s u", s=2),
            ).then_inc(dwc, 16)
            act.wait_ge(sini, 1)
            # warm the activation table off the critical path
            nc.scalar.activation(out=ap(t["wscr"]), in_=ap(t["zcol"]),
                                 func=Act.Relu, bias=t["zcol"].ap()[:, 0:1])
            act.wait_ge(spe, 1)              # bc = ones x b broadcast done
            nc.scalar.activation(out=ap(t["z0"]), in_=bc.ap(), func=Act.Relu,
                                 bias=t["zcol"].ap()[:, 0:1],
                                 accum_out=t["z0sum"].ap()[:, 0:1]
                                 ).then_inc(sz0, 1)
            act.wait_ge(dxb, 16)
            act.wait_ge(sdve, 1)             # k
            nc.scalar.activation(out=t["xcat"].ap()[:, 0:C],
                                 in_=t["xb_t"].ap()[:, 0:C], func=Act.Copy,
                                 scale=t["k"].ap()[:, 0:1]
                                 ).then_inc(sact, 1)                   # ->1
            act.wait_ge(sdve, 3)             # asm1
            act.wait_ge(sact, 1)             # xk visible (self)
            nc.scalar.activation(out=t["xcat"].ap()[:, C:2 * C],
                                 in_=t["xcat"].ap()[:, 0:C], func=Act.Copy,
                                 scale=t["asm1"].ap()[:, 0:1]
                                 ).then_inc(sact, 1)                   # ->2
            act.wait_ge(spool, 5)            # h0 (all of iota..h1)
            nc.scalar.activation(out=ap(t["z0h"]), in_=ap(t["z0"]),
                                 func=Act.Copy, scale=t["h0"].ap()[:, 0:1]
                                 ).then_inc(sact, 1)                   # ->3
            act.wait_ge(sdve, 10)            # rn
            act.wait_ge(sact, 3)             # xcat fully visible
            nc.scalar.activation(out=t["out_t"].ap()[:, U:OUTF],
                                 in_=ap(t["xcat"]), func=Act.Copy,
                                 scale=t["rn"].ap()[:, 0:1]
                                 ).then_inc(sfin, 1)

        @block.tensor
        def _(pe):
            pe.wait_ge(sini, 1)              # ones1 ready
            pe.wait_ge(db, 16)               # b landed
            nc.tensor.matmul(bc.ap(), lhsT=t["ones1"].ap(),
                             rhs=ap(t["brow_t"]), start=True,
                             stop=True).then_inc(spe, 1)    # ->1 (bc ready)
            pe.wait_ge(dxb, 16)
            pe.wait_ge(dwc, 16)
            nc.tensor.matmul(mm.ap(), lhsT=t["xb_t"].ap()[:, C:2 * C],
                             rhs=t["wcat_t"].ap(), start=True,
                             stop=True).then_inc(spe, 1)    # ->2 (mm ready)

        @block.vector
        def _(dve):
            dve.wait_ge(dadj, 16)
            nc.vector.reduce_sum(ap(t["k"]), ap(t["adj_t"]),
                                 axis=AX).then_inc(sdve, 1)            # ->1
            dve.wait_ge(spool, 2)            # iota + xidx_f
            nc.vector.scalar_tensor_tensor(
                out=ap(t["scr"]), in0=ap(t["iota_f"]),
                scalar=t["xidx_f"].ap()[:, 0:1], in1=ap(t["adj_t"]),
                op0=Alu.is_equal, op1=Alu.mult,
                accum_out=t["a_sel"].ap()[:, 0:1]).then_inc(sdve, 1)   # ->2
            dve.wait_ge(sdve, 2)             # a_sel accum lands async
            nc.vector.tensor_scalar(out=ap(t["asm1"]), in0=ap(t["a_sel"]),
                                    scalar1=-1.0, scalar2=None,
                                    op0=Alu.add).then_inc(sdve, 1)     # ->3
            dve.wait_ge(spe, 2)              # mm = [u | v]
            dve.wait_ge(sdve, 3)             # asm1 visible
            nc.vector.tensor_scalar(out=ap(t["t_sb"]),
                                    in0=mm.ap()[:, U:2 * U],
                                    scalar1=t["asm1"].ap()[:, 0:1],
                                    scalar2=None,
                                    op0=Alu.mult).then_inc(sdve, 1)    # ->4
            dve.wait_ge(sdve, 4)             # t_sb visible
            nc.vector.tensor_add(ap(t["zz"]), ap(t["t_sb"]),
                                 mm.ap()[:, 0:U]).then_inc(sdve, 1)    # ->5
            dve.wait_ge(sdve, 5)             # zz visible
            dve.wait_ge(spe, 2)              # bc ready
            nc.vector.tensor_add(ap(t["zzb"]), ap(t["zz"]),
                                 bc.ap()).then_inc(sdve, 1)            # ->6
            dve.wait_ge(sdve, 6)             # zzb visible
            nc.vector.tensor_scalar(out=ap(t["z1"]), in0=ap(t["zzb"]),
                                    scalar1=0.0, scalar2=None, op0=Alu.max,
                                    op1=Alu.add,
                                    accum_out=t["z1sum"].ap()[:, 0:1]
                                    ).then_inc(sdve, 1)                # ->7
            dve.wait_ge(sdve, 7)             # z1sum accum landed
            nc.vector.tensor_scalar(out=ap(t["s1"]), in0=ap(t["z1sum"]),
                                    scalar1=0.0, scalar2=None,
                                    op0=Alu.is_gt).then_inc(sdve, 1)   # ->8
            dve.wait_ge(spool, 7)            # t2
            dve.wait_ge(sdve, 8)             # s1 visible
            nc.vector.scalar_tensor_tensor(
                out=ap(t["nn"]), in0=ap(t["k"]),
                scalar=t["s1"].ap()[:, 0:1], in1=ap(t["t2"]),
                op0=Alu.mult, op1=Alu.add).then_inc(sdve, 1)           # ->9
            dve.wait_ge(sdve, 9)             # nn visible
            nc.vector.reciprocal(ap(t["rn"]),
                                 ap(t["nn"])).then_inc(sdve, 1)        # ->10
            dve.wait_ge(sact, 3)             # z0h
            nc.vector.scalar_tensor_tensor(
                out=t["out_t"].ap()[:, 0:U], in0=ap(t["z1"]),
                scalar=t["h1"].ap()[:, 0:1], in1=ap(t["z0h"]),
                op0=Alu.mult, op1=Alu.max).then_inc(sfin, 1)

    nc.compile()
    return nc


def get_nc():
    if "nc" not in _CACHE:
        _CACHE["nc"] = _build_nc()
    return _CACHE["nc"]


def make_in_maps(inputs, adj_matrix, xidx, w, b):
    """Shard full inputs into per-core input maps (128 (b,i) rows per core)."""
    x_flat = np.asarray(inputs, dtype=np.float32).reshape(B * N, C)
    adj_flat = np.ascontiguousarray(
        np.asarray(adj_matrix, dtype=np.float32).reshape(B * N, N))
    xidx_flat = np.ascontiguousarray(
        np.asarray(xidx, dtype=np.int32).reshape(B * N, 1))
    w_full = np.ascontiguousarray(np.asarray(w, dtype=np.float32)[0])
    b_full = np.ascontiguousarray(
        np.asarray(b, dtype=np.float32).reshape(1, U))

    in_maps = []
    for c in range(NCORES):
        rows = slice(c * P, (c + 1) * P)
        x_slab = x_flat[rows]
        in_maps.append({
            "adj": adj_flat[rows],
            "xboth": np.ascontiguousarray(
                np.concatenate([x_slab, x_slab.T], axis=1)),
            "xidx": xidx_flat[rows],
            "w": w_full,
            "b": b_full,
        })
    return in_maps


def kernel(inputs, adj_matrix, xidx, w, b, _trace=False):
    from concourse.bass_utils import run_bass_kernel_spmd

    nc = get_nc()
    in_maps = make_in_maps(inputs, adj_matrix, xidx, w, b)
    res = run_bass_kernel_spmd(nc, in_maps, list(range(NCORES)),
                               trace=_trace)
    out = np.concatenate([res.results[c]["out"] for c in range(NCORES)],
                         axis=0)
    out = out.reshape(B, N, OUTF).astype(np.float32)
    if _trace:
        _CACHE["last_results"] = res
    return out



# revision 18
# speedup vs baseline: 1.4413x; 1.4413x over previous
"""Trainium2 Bass kernel for nn_EdgeConvolution (gnn_message_passing).

Math
----
Reference (B=2, N=512, C=128, U=128), adj binary {0,1}:
  masked[b,i,j,:]  = adj[b,i,j] * x[b,i,:]
  a_sel[b,i]       = adj[b,i, xidx[b,i]]
  edging[b,i,j,:]  = adj[b,i,j] * [ x_i | (a_sel_i - 1)*x_i ]      (adj^2 = adj)
  out[b,i,j,:]     = relu(adj*(u_i + (a_sel_i-1)*v_i) + b),  u = x@W1, v = x@W2
Over j there are only two row values per (b,i):
  zzb_i = u_i + (a_sel_i-1)*v_i + b    (edges with adj=1, count k_i)
  z0    = relu(b)                       (edges with adj=0, count N-k_i)
  maxp_i   = max(h1_i*zzb_i, h0_i*z0)  elementwise, h1=[k>0], h0=[k<N]
             (relu is absorbed: z0>=0, and h*zzb with max against >=0)
  n_i      = k_i*[any zzb_i>0] + (N-k_i)*[any b>0]
  avg_i    = [ k_i*x_i | k_i*(a_sel_i-1)*x_i ] / n_i
Per-core slab: 128 of the 1024 (b,i) rows; w/b replicated.

Implementation: raw Bass. Inputs are host-packed into TWO fp16 DMAs
(adj {0,1} and iota/xidx values are exact in fp16; x/w quantization
~1e-3 rel err, well under the 2e-2 gate):
  adj16 [128,512]  on the SP HWDGE ring
  aux16 [128,644] = [xT | x | wcat | b_rep | xidx | pad] on the ACT ring
so every transfer has >=1KB per-partition descriptors (the previous
version's 4B-descriptor xidx DMA and 512B-strided w DMA each cost ~5us
of completion latency). PE does one fp16 matmul x@[W1|W2] (bias folded
later, not via a K=1 matmul). ACT computes k (copy+accum over adj) in
parallel with DVE's a_sel pass (iota==xidx dot adj). DVE chain uses
tensor_tensor_reduce to fuse the bias add with the any()>0 max-reduce.
Outputs leave via two DMAs (maxp half from SP, avg half from ACT ring).
6 semaphores total (sem init/teardown storms scale with sem count).
"""

import numpy as np

B, N, C, U = 2, 512, 128, 128
P = 128          # rows (b,i) per core == SBUF partitions
NCORES = 8
OUTF = U + 2 * C  # 384
FAUX = 2 * C + 3 * U + 4  # 644: xT(C) | x(C) | wcat(2U) | b(U) | xidx(1) | pad(3)

_CACHE: dict = {}


def _build_nc():
    import concourse.bacc as bacc
    import concourse.bass as bass
    import concourse.mybir as mybir

    f32 = mybir.dt.float32
    f16 = mybir.dt.float16
    Alu = mybir.AluOpType
    AX = mybir.AxisListType.X
    Act = mybir.ActivationFunctionType

    nc = bacc.Bacc("TRN2", target_bir_lowering=False, debug=False,
                   num_devices=NCORES)

    adj_d = nc.dram_tensor("adj", [P, N], f16, kind="ExternalInput")
    aux_d = nc.dram_tensor("aux", [P, FAUX], f16, kind="ExternalInput")
    out_d = nc.dram_tensor("out", [P, OUTF], f32, kind="ExternalOutput")

    # aux column layout (fp16 units)
    A_XT = 0            # [0,128)   xT: lhsT for matmul
    A_X = C             # [128,256) x rows
    A_W = 2 * C         # [256,512) wcat: [c, (W1[c,:] | W2[c,:])]
    A_B = 2 * C + 2 * U  # [512,640) b replicated to all partitions
    A_IDX = A_B + U     # [640,641) xidx as fp16 (exact, < 2048)

    ctx_tensors = [
        ("adj_t", [P, N], f16), ("aux_t", [P, FAUX], f16),
        ("iota_t", [P, N], f16),
        ("scr_v", [P, N], f16),   # a_sel pass throwaway output
        ("scr_k", [P, N], f16),   # k pass throwaway output
        ("z0", [P, C], f32), ("z0h", [P, C], f32), ("zzb2", [P, U], f32),
        ("ub", [P, U], f32), ("b32", [P, U], f32),
        ("xcat", [P, 2 * C], f16),
        ("out_t", [P, OUTF], f32),
        ("a_sel", [P, 1], f32), ("asm1", [P, 1], f32),
        ("z0sum", [P, 1], f32),
        ("zmax", [P, 1], f32), ("s1", [P, 1], f32), ("k", [P, 1], f32),
        ("nk", [P, 1], f32), ("t2", [P, 1], f32), ("s0", [P, 1], f32),
        ("h0", [P, 1], f32), ("h1", [P, 1], f32),
        ("nn", [P, 1], f32), ("rn", [P, 1], f32),
    ]

    from contextlib import ExitStack
    with ExitStack() as ctx:
        t = {}
        for name, shape, dt in ctx_tensors:
            t[name] = ctx.enter_context(nc.sbuf_tensor(name, shape, dt))
        mm = ctx.enter_context(nc.psum_tensor("mm", [P, 2 * U], f32))

        da = ctx.enter_context(nc.semaphore("da"))    # adj + out DMAs
        dx = ctx.enter_context(nc.semaphore("dx"))    # aux DMA
        sv = ctx.enter_context(nc.semaphore("sv"))    # DVE events
        sa = ctx.enter_context(nc.semaphore("sa"))    # ACT events
        sp = ctx.enter_context(nc.semaphore("sp"))    # Pool events
        se = ctx.enter_context(nc.semaphore("se"))    # PE events

        block = ctx.enter_context(nc.Block())

        ap = lambda h: h.ap()

        def aux_t_b(t):
            return t["aux_t"].ap()[:, A_B:A_B + U]

        @block.gpsimd
        def _(pool):
            nc.gpsimd.iota(ap(t["iota_t"]), pattern=[[1, N]], base=0,
                           channel_multiplier=0,
                           allow_small_or_imprecise_dtypes=True
                           ).then_inc(sp, 1)                           # ->1
            pool.wait_ge(dx, 16)
            # b32 = b (f16 -> f32) for the PSUM-safe ub add on DVE
            nc.gpsimd.tensor_copy(ap(t["b32"]),
                                  aux_t_b(t)).then_inc(sp, 1)          # ->2
            pool.wait_ge(sa, 2)              # z0sum
            nc.gpsimd.tensor_scalar(out=ap(t["s0"]), in0=ap(t["z0sum"]),
                                    scalar1=0.0, scalar2=None,
                                    op0=Alu.is_gt).then_inc(sp, 1)     # ->3
            nc.gpsimd.tensor_scalar(out=ap(t["h1"]), in0=ap(t["k"]),
                                    scalar1=0.0, scalar2=None,
                                    op0=Alu.is_gt).then_inc(sp, 1)     # ->4
            nc.gpsimd.tensor_scalar(out=ap(t["nk"]), in0=ap(t["k"]),
                                    scalar1=-1.0, scalar2=float(N),
                                    op0=Alu.mult,
                                    op1=Alu.add).then_inc(sp, 1)       # ->5
            pool.wait_ge(sp, 5)              # nk visible (self)
            nc.gpsimd.tensor_mul(ap(t["t2"]), ap(t["nk"]),
                                 ap(t["s0"])).then_inc(sp, 1)          # ->6
            nc.gpsimd.tensor_scalar(out=ap(t["h0"]), in0=ap(t["nk"]),
                                    scalar1=0.0, scalar2=None,
                                    op0=Alu.is_gt).then_inc(sp, 1)     # ->7

        @block.sync
        def _(sync):
            sync.dma_start(ap(t["adj_t"]), adj_d.ap()).then_inc(da, 16)
            sync.wait_ge(sv, 9)              # maxp half written
            sync.dma_start(out_d.ap()[:, 0:U],
                           t["out_t"].ap()[:, 0:U]).then_inc(da, 16)
            sync.wait_ge(da, 48)             # adj + both out DMAs done

        @block.scalar
        def _(act):
            act.dma_start(ap(t["aux_t"]), aux_d.ap()).then_inc(dx, 16)
            # warm any ACT table load off the critical path (garbage in,
            # scratch out)
            nc.scalar.activation(out=t["scr_k"].ap()[0:1, 0:1],
                                 in_=t["scr_k"].ap()[0:1, 0:1],
                                 func=Act.Relu)
            act.wait_ge(da, 16)              # adj landed
            act.wait_ge(dx, 16)              # aux landed (b for z0)
            nc.scalar.activation(out=ap(t["scr_k"]), in_=ap(t["adj_t"]),
                                 func=Act.Copy,
                                 accum_out=t["k"].ap()[:, 0:1]
                                 ).then_inc(sa, 1)                     # ->1
            # z0 = relu(b); z0sum>0 <=> any(b>0)
            nc.scalar.activation(out=ap(t["z0"]), in_=aux_t_b(t),
                                 func=Act.Relu,
                                 accum_out=t["z0sum"].ap()[:, 0:1]
                                 ).then_inc(sa, 1)                     # ->2
            act.wait_ge(sa, 2)               # k/z0 accums visible (self)
            act.wait_ge(sv, 2)               # asm1
            nc.scalar.activation(out=t["xcat"].ap()[:, 0:C],
                                 in_=t["aux_t"].ap()[:, A_X:A_X + C],
                                 func=Act.Copy, scale=t["k"].ap()[:, 0:1]
                                 ).then_inc(sa, 1)                     # ->3
            act.wait_ge(sa, 3)               # xcat0 visible (self)
            nc.scalar.activation(out=t["xcat"].ap()[:, C:2 * C],
                                 in_=t["xcat"].ap()[:, 0:C],
                                 func=Act.Copy, scale=t["asm1"].ap()[:, 0:1]
                                 ).then_inc(sa, 1)                     # ->4
            act.wait_ge(sp, 7)               # h0 (and all pool scalars)
            nc.scalar.activation(out=ap(t["z0h"]), in_=ap(t["z0"]),
                                 func=Act.Copy, scale=t["h0"].ap()[:, 0:1]
                                 ).then_inc(sa, 1)                     # ->5
            act.wait_ge(sa, 4)               # xcat1 visible (self)
            act.wait_ge(sv, 8)               # rn
            nc.scalar.activation(out=t["out_t"].ap()[:, U:OUTF],
                                 in_=ap(t["xcat"]), func=Act.Copy,
                                 scale=t["rn"].ap()[:, 0:1]
                                 ).then_inc(sa, 1)                     # ->6
            act.wait_ge(sa, 6)               # final write visible to DMA
            act.dma_start(out_d.ap()[:, U:OUTF],
                          t["out_t"].ap()[:, U:OUTF]).then_inc(da, 16)

        @block.tensor
        def _(pe):
            pe.wait_ge(dx, 16)
            nc.tensor.matmul(mm.ap(),
                             lhsT=t["aux_t"].ap()[:, A_XT:A_XT + C],
                             rhs=t["aux_t"].ap()[:, A_W:A_W + 2 * U],
                             start=True, stop=True).then_inc(se, 1)    # ->1

        @block.vector
        def _(dve):
            dve.wait_ge(da, 16)
            dve.wait_ge(dx, 16)              # aux landed (xidx scalar)
            dve.wait_ge(sp, 1)               # iota
            # a_sel = sum_j adj[i,j] * (iota[j] == xidx[i])
            nc.vector.scalar_tensor_tensor(
                out=ap(t["scr_v"]), in0=ap(t["iota_t"]),
                scalar=t["aux_t"].ap()[:, A_IDX:A_IDX + 1],
                in1=ap(t["adj_t"]),
                op0=Alu.is_equal, op1=Alu.mult,
                accum_out=t["a_sel"].ap()[:, 0:1]).then_inc(sv, 1)     # ->1
            dve.wait_ge(sv, 1)               # a_sel accum lands async
            nc.vector.tensor_scalar(out=ap(t["asm1"]), in0=ap(t["a_sel"]),
                                    scalar1=-1.0, scalar2=None,
                                    op0=Alu.add).then_inc(sv, 1)       # ->2
            dve.wait_ge(se, 1)               # mm = [u | v]
            dve.wait_ge(sp, 2)               # b32
            nc.vector.tensor_add(ap(t["ub"]), mm.ap()[:, 0:U],
                                 ap(t["b32"])).then_inc(sv, 1)         # ->3
            dve.wait_ge(sv, 3)               # ub + asm1 visible (self)
            # zzb = asm1 * v + (u + b)
            nc.vector.scalar_tensor_tensor(
                out=ap(t["zzb2"]), in0=mm.ap()[:, U:2 * U],
                scalar=t["asm1"].ap()[:, 0:1], in1=ap(t["ub"]),
                op0=Alu.mult, op1=Alu.add).then_inc(sv, 1)             # ->4
            dve.wait_ge(sv, 4)               # zzb2 visible (self)
            nc.vector.reduce_max(t["zmax"].ap()[:, 0:1], ap(t["zzb2"]),
                                 axis=AX).then_inc(sv, 1)              # ->5
            dve.wait_ge(sv, 5)               # zmax visible (self)
            dve.wait_ge(sa, 1)               # k
            dve.wait_ge(sp, 6)               # t2
            # s1k = (zmax > 0) * k
            nc.vector.scalar_tensor_tensor(
                out=ap(t["s1"]), in0=ap(t["zmax"]),
                scalar=0.0, in1=ap(t["k"]),
                op0=Alu.is_gt, op1=Alu.mult).then_inc(sv, 1)           # ->6
            dve.wait_ge(sv, 6)               # s1k visible (self)
            nc.vector.tensor_add(ap(t["nn"]), ap(t["s1"]),
                                 ap(t["t2"])).then_inc(sv, 1)          # ->7
            dve.wait_ge(sv, 7)               # nn visible (self)
            nc.vector.reciprocal(ap(t["rn"]),
                                 ap(t["nn"])).then_inc(sv, 1)          # ->8
            dve.wait_ge(sa, 5)               # z0h (h1 lands before it)
            nc.vector.scalar_tensor_tensor(
                out=t["out_t"].ap()[:, 0:U], in0=ap(t["zzb2"]),
                scalar=t["h1"].ap()[:, 0:1], in1=ap(t["z0h"]),
                op0=Alu.mult, op1=Alu.max).then_inc(sv, 1)             # ->9

    nc.compile()
    return nc


def get_nc():
    if "nc" not in _CACHE:
        _CACHE["nc"] = _build_nc()
    return _CACHE["nc"]


def make_in_maps(inputs, adj_matrix, xidx, w, b):
    """Shard full inputs into per-core input maps (128 (b,i) rows per core)."""
    x_flat = np.asarray(inputs, dtype=np.float32).reshape(B * N, C)
    adj16 = np.ascontiguousarray(
        np.asarray(adj_matrix).reshape(B * N, N).astype(np.float16))
    xidx16 = np.asarray(xidx, dtype=np.float32).reshape(B * N, 1) \
        .astype(np.float16)
    w_full = np.asarray(w, dtype=np.float32)[0]          # [2C, U]
    # wcat[c, :] = [W1[c, :] | W2[c, :]]
    wcat16 = np.concatenate([w_full[:C], w_full[C:]], axis=1) \
        .astype(np.float16)                               # [C, 2U]
    b16 = np.asarray(b, dtype=np.float32).reshape(1, U).astype(np.float16)

    in_maps = []
    for c in range(NCORES):
        rows = slice(c * P, (c + 1) * P)
        x_slab = x_flat[rows]
        aux = np.zeros((P, FAUX), dtype=np.float16)
        aux[:, 0:C] = x_slab.T                            # xT (lhsT)
        aux[:, C:2 * C] = x_slab                          # x
        aux[:, 2 * C:2 * C + 2 * U] = wcat16              # wcat
        aux[:, 2 * C + 2 * U:2 * C + 2 * U + U] = b16     # b replicated
        aux[:, 2 * C + 2 * U + U:2 * C + 2 * U + U + 1] = xidx16[rows]
        in_maps.append({
            "adj": adj16[rows],
            "aux": np.ascontiguousarray(aux),
        })
    return in_maps


def kernel(inputs, adj_matrix, xidx, w, b, _trace=False):
    from concourse.bass_utils import run_bass_kernel_spmd

    nc = get_nc()
    in_maps = make_in_maps(inputs, adj_matrix, xidx, w, b)
    res = run_bass_kernel_spmd(nc, in_maps, list(range(NCORES)),
                               trace=_trace)
    out = np.concatenate([res.results[c]["out"] for c in range(NCORES)],
                         axis=0)
    out = out.reshape(B, N, OUTF).astype(np.float32)
    if _trace:
        _CACHE["last_results"] = res
    return out
